# revision 4
# baseline (speedup 1.0000x reference)
"""ROI max-pooling on 8 trn2 cores via dma_gather + DVE max-reduce.

v2 strategy (hardcoded for feats[8,128,128,256] f32, rois[512,5] i32, 7x7):
  - Sort ROIs by (batch, -h); form 32 chunks of 16 ROIs (each spans <= 2
    consecutive batches so gather indices fit int16 over a 2-batch feats
    slice of [32768, 256]).
  - Deal chunks to (core, slot) by descending workload (snake order) so the
    8 chunks sharing a slot have similar row-replica profiles.
  - Within a chunk, sort the 784 (roi, bin) samples by row-extent bh
    descending.  Gather t (t-th row replica) then only needs the first
    G[c][t] = max-over-cores ceil(n_t/128) partition blocks; samples with
    bh <= t inside that prefix fetch their last valid row (duplicate,
    harmless under max).  Shrinks gather DMA ~40% vs fetching 5 replicas
    for every sample.
  - Per gather: 5 col replicas per sample (slot = g*5 + s), clamped to the
    last valid col.  DVE: tensor_reduce max over s -> [128, G, 256], then
    elementwise max into acc[:, :G].
  - Host reassembles [512,7,7,256] undoing both sorts.
"""

import os

import numpy as np

B, H, W, C = 8, 128, 128, 256
N = 512
PH = PW = 7
MAXR = 32
NCORES = 8
NCH = 4                # chunks per core
RPCH = 16              # ROIs per chunk
NBIN = PH * PW         # 49
NSAMP = RPCH * NBIN    # 784 samples per chunk
NPOS = 896             # 7 partition blocks of 128
S = 5                  # col replicas (max bin width)
T = 5                  # row replicas (max bin height)
NEG_INF = -1e30

_cache = {}


def _build_nc(G):
    """G: tuple of NCH tuples, G[c][t] = partition blocks for gather (c,t)."""
    import concourse.bacc as bacc
    import concourse.mybir as mybir
    import concourse.tile as tile
    from concourse.library_config import mlp

    f32 = mybir.dt.float32
    nc = bacc.Bacc("TRN2")
    feats_d = [
        nc.dram_tensor(f"feats{c}", [2 * H * W, C], f32, kind="ExternalInput")
        for c in range(NCH)
    ]
    totw = 40 * sum(g for gs in G for g in gs)
    idx_d = nc.dram_tensor("gidx", [128, totw], mybir.dt.int16,
                           kind="ExternalInput")
    out_d = nc.dram_tensor("out", [NCH, NPOS, C], f32, kind="ExternalOutput")

    with tile.TileContext(nc) as tc:
        with tc.tile_pool(name="idxp", bufs=1) as ipool, \
             tc.tile_pool(name="accp", bufs=2) as apool, \
             tc.tile_pool(name="gp", bufs=2) as gpool, \
             tc.tile_pool(name="rp", bufs=2) as rpool:
            idx_sb = ipool.tile([128, totw], mybir.dt.int16)
            nc.sync.dma_start(idx_sb[:], idx_d[:])
            nc.gpsimd.load_library(mlp)
            off = 0
            for c in range(NCH):
                acc = apool.tile([128, PH, C], f32)
                for t in range(T):
                    g = G[c][t]
                    if g == 0:
                        continue
                    # HW limit: one dma_gather handles at most 1024 idxs,
                    # so issue one 640-idx gather per 128-sample block.
                    dst = gpool.tile([128, g * S, C], f32)
                    for blk in range(g):
                        nc.gpsimd.dma_gather(
                            dst[:, blk * S:(blk + 1) * S, :],
                            feats_d[c][:, :],
                            idx_sb[:, off:off + 40],
                            S * 128,
                            S * 128,
                            C,
                        )
                        off += 40
                    red = acc if t == 0 else rpool.tile([128, g, C], f32)
                    nc.vector.tensor_reduce(
                        out=red[:, 0:g, :],
                        in_=dst[:].rearrange("p (g s) c -> p g c s", s=S),
                        axis=mybir.AxisListType.X,
                        op=mybir.AluOpType.max,
                    )
                    if t > 0:
                        nc.vector.tensor_tensor(
                            out=acc[:, 0:g, :], in0=acc[:, 0:g, :],
                            in1=red[:, 0:g, :], op=mybir.AluOpType.max,
                        )
                nc.sync.dma_start(
                    out=out_d[c].rearrange("(g p) c -> p g c", p=128),
                    in_=acc[:],
                )
    nc.compile()
    return nc


def _bin_starts(sz, nbins):
    i = np.arange(nbins, dtype=np.int64)
    return (i[None, :] * sz[:, None] + nbins - 1) // nbins


def kernel(feats, rois, pool_height, pool_width):
    assert int(pool_height) == PH and int(pool_width) == PW
    assert feats.shape == (B, H, W, C) and rois.shape == (N, 5)

    b = rois[:, 0].astype(np.int64)
    x1 = rois[:, 1].astype(np.int64)
    y1 = rois[:, 2].astype(np.int64)
    y2 = rois[:, 4].astype(np.int64)
    x2 = rois[:, 3].astype(np.int64)
    h = np.clip(y2 - y1 + 1, 1, MAXR)
    w = np.clip(x2 - x1 + 1, 1, MAXR)

    r0 = _bin_starts(h, PH)                                   # [N,7]
    bh = np.concatenate([r0[:, 1:], h[:, None]], 1) - r0      # [N,7]
    c0 = _bin_starts(w, PW)
    bw = np.concatenate([c0[:, 1:], w[:, None]], 1) - c0

    order = np.lexsort((-h, b))
    chunks = order.reshape(NCORES * NCH, RPCH)                # 32 x 16

    # per-chunk row-replica profile: n_t = #samples with bh > t
    nt = np.zeros((NCORES * NCH, T), np.int64)
    for m in range(NCORES * NCH):
        cb = bh[chunks[m]]                                    # [16,7]
        for t in range(T):
            nt[m, t] = PW * np.count_nonzero(cb > t)
    rank = np.argsort(-nt.sum(1), kind="stable")
    assign = np.empty((NCORES, NCH), np.int64)                # chunk id
    for rho, m in enumerate(rank):
        c = rho // NCORES
        k = rho % NCORES if c % 2 == 0 else NCORES - 1 - rho % NCORES
        assign[k, c] = m
    Gtab = tuple(
        tuple(int(max((nt[assign[k, c], t] + 127) // 128 for k in range(NCORES)))
              for t in range(T))
        for c in range(NCH)
    )
    assert all(g[0] == PH for g in Gtab)

    feats_flat = np.ascontiguousarray(feats).reshape(B * H * W, C)
    ss = np.arange(S)

    in_maps = []
    pos_maps = np.empty((NCORES, NCH, NSAMP), np.int64)       # pi per chunk
    for k in range(NCORES):
        fm = {}
        idx_cols = []
        for c in range(NCH):
            sel = chunks[assign[k, c]]                        # 16 roi ids
            base = int(min(b[sel].min(), B - 2))
            assert b[sel].max() - base <= 1, "chunk spans >2 batches"
            fm[f"feats{c}"] = feats_flat[base * H * W:(base + 2) * H * W]

            cbh = np.maximum(bh[sel], 1)                      # [16,7]
            cbw = np.maximum(bw[sel], 1)
            cols = x1[sel][:, None, None] + np.minimum(
                c0[sel][:, :, None] + ss[None, None, :],
                (c0[sel] + cbw - 1)[:, :, None])              # [16,7bj,S]
            cols = np.clip(cols, 0, W - 1)
            boff = (b[sel] - base) * (H * W)                  # [16]

            samp_bh = np.broadcast_to(cbh[:, :, None],
                                      (RPCH, PH, PW)).reshape(NSAMP)
            pi = np.argsort(-samp_bh, kind="stable")
            pos_maps[k, c] = pi

            for t in range(T):
                g = Gtab[c][t]
                if g == 0:
                    continue
                rows = y1[sel][:, None] + np.minimum(
                    r0[sel] + t, r0[sel] + cbh - 1)           # [16,7bi]
                rows = np.clip(rows, 0, H - 1)
                pix = (boff[:, None, None, None]
                       + rows[:, :, None, None] * W
                       + cols[:, None, :, :])                 # [16,7bi,7bj,S]
                pflat = pix.reshape(NSAMP, S)[pi]             # sorted samples
                npad = g * 128 - NSAMP
                if npad > 0:
                    pflat = np.concatenate(
                        [pflat, np.broadcast_to(pflat[:1], (npad, S))])
                else:
                    pflat = pflat[:g * 128]
                assert pflat.min() >= 0 and pflat.max() <= 32767
                flat = pflat.reshape(g, 128, S).transpose(0, 2, 1).reshape(-1)
                wrapped = flat.reshape(g * S * 8, 16).T       # [16, g*40]
                idx_cols.append(np.tile(wrapped.astype(np.int16), (8, 1)))
        fm["gidx"] = np.ascontiguousarray(np.concatenate(idx_cols, axis=1))
        in_maps.append(fm)

    if Gtab not in _cache:
        _cache[Gtab] = _build_nc(Gtab)
    nc = _cache[Gtab]

    from concourse.bass_utils import run_bass_kernel_spmd
    trace = os.environ.get("ROI_TRACE") == "1"
    res = run_bass_kernel_spmd(nc, in_maps, core_ids=list(range(NCORES)),
                               trace=trace)
    _cache["last_results"] = res

    pooled = np.empty((N, NBIN, C), np.float32)
    for k in range(NCORES):
        for c in range(NCH):
            vals = res.results[k]["out"][c]                   # [896, 256]
            inv = np.empty(NSAMP, np.int64)
            inv[pos_maps[k, c]] = np.arange(NSAMP)
            pooled[chunks[assign[k, c]]] = vals[inv].reshape(RPCH, NBIN, C)

    empty = (bh <= 0)[:, :, None] | (bw <= 0)[:, None, :]     # [N,7,7]
    pooled = pooled.reshape(N, PH, PW, C)
    pooled[empty] = 0.0
    return pooled.astype(np.float32)
